# revision 1
# baseline (speedup 1.0000x reference)
"""GCN2Net Trainium2 kernel (8-core SPMD), v5.

v2 -> v3:
- Rotated per-cell chunk caps PAT[(s - t) % 4] (default [5,5,4,4]): 18 chunks
  per tile instead of 20 (-10% gather packets), enforced by a cell-aware
  vector bin-pack on the host.
- Seg-3 AllGather split into 5 per-group parts issued as their tiles finish,
  so the layer-boundary collective is ~1.6MB instead of 6.5MB.
- Seg-3 gather calls deferred by one group so their wait on the (split)
  AllGather never stalls the gather queue.
"""
import math
import os
import numpy as np

P = 128
N_CORES = 8
N_NODES = 100000
IN_DIM = 512
HID = 128
N_LAYERS = 8
ALPHA = 0.1
THETA = 0.5

T_PER_CORE = 100
SHARD = T_PER_CORE * P            # 12800
NSEG = 4
TPS = T_PER_CORE // NSEG          # 25 tiles per seg
SEG_ROWS = N_CORES * TPS * P      # 25600 (< 32768)
G_TILES = 5                       # tiles per gather group
N_GROUPS = T_PER_CORE // G_TILES  # 20
GPS = TPS // G_TILES              # 5 groups per seg
IN_PAD = 640                      # x rows 0-511, b-row 512, zero pad
N_QUEUES = int(os.environ.get("GCN_QUEUES", "4"))
PAT = [int(c) for c in os.environ.get("GCN_PAT", "5444")]
assert len(PAT) == NSEG

BETAS = [math.log(THETA / (i + 1) + 1.0) for i in range(N_LAYERS)]

# static chunk geometry
CAPS = [[PAT[(s - t) % NSEG] for s in range(NSEG)] for t in range(T_PER_CORE)]
K_T = sum(CAPS[0])                                  # chunks per tile (const)
assert all(sum(CAPS[t]) == K_T for t in range(T_PER_CORE))
KTOT = T_PER_CORE * K_T
# chunk base of (t, s) within tile t's chunk list
CELL_BASE = [[sum(CAPS[t][:s]) for s in range(NSEG)] for t in range(T_PER_CORE)]
# chunks per gather call (g, s)
CALL_CH = [[sum(CAPS[g * G_TILES + i][s] for i in range(G_TILES))
            for s in range(NSEG)] for g in range(N_GROUPS)]
# tile offset (in chunks) within call (g, s)
TILE_OFF = [[[sum(CAPS[g * G_TILES + j][s] for j in range(i))
              for i in range(G_TILES)] for s in range(NSEG)]
            for g in range(N_GROUPS)]
# idx-stream block offsets (in idxs), canonical (g, s) order
CALL_OFF = {}
_off = 0
for _g in range(N_GROUPS):
    for _s in range(NSEG):
        CALL_OFF[(_g, _s)] = _off
        _off += CALL_CH[_g][_s] * P
IDX_TOT = _off                                       # == KTOT * P
DEFER3 = 3                        # defer seg-3 gather calls by this many groups


# ----------------------------------------------------------------- host prep
def _wrap16(stream):
    n = stream.shape[0]
    wrap = stream.reshape(n // 16, 16).T.astype(np.int16)
    out = np.zeros((P, n // 16), dtype=np.int16)
    for rg in range(8):
        out[rg * 16:(rg + 1) * 16] = wrap
    return out


def _preprocess(x, edge_index):
    import heapq

    src = np.asarray(edge_index[0], dtype=np.int64)
    dst = np.asarray(edge_index[1], dtype=np.int64)

    deg = np.bincount(dst, minlength=N_NODES).astype(np.float64) + 1.0
    dinv = 1.0 / np.sqrt(deg)
    sqrtdeg = np.sqrt(deg)
    cnt = np.bincount(dst, minlength=N_NODES).astype(np.int64)

    n_tiles_total = N_CORES * T_PER_CORE
    order = np.argsort(-cnt, kind="stable")

    band_of = np.empty(N_NODES, dtype=np.int64)
    band_of[order] = np.arange(N_NODES, dtype=np.int64) % NSEG

    prof = np.zeros((N_NODES, NSEG), dtype=np.int64)
    np.add.at(prof, (dst, band_of[src]), 1)

    caps_row = np.array(
        [[CAPS[t % T_PER_CORE][s] * P for s in range(NSEG)]
         for t in range(n_tiles_total)], dtype=np.int64)

    tile_of = np.empty(N_NODES, dtype=np.int32)
    slot_of = np.empty(N_NODES, dtype=np.int32)
    tile_fill = np.zeros(n_tiles_total, dtype=np.int32)
    cell_load = np.zeros((n_tiles_total, NSEG), dtype=np.int64)
    for b in range(NSEG):
        tiles_b = np.array([c * T_PER_CORE + b * TPS + j
                            for c in range(N_CORES) for j in range(TPS)])
        caps_b = caps_row[tiles_b].astype(np.float64)
        load_b = np.zeros((len(tiles_b), NSEG))
        fill_b = np.zeros(len(tiles_b))
        nodes_b = order[band_of[order] == b]
        for v in nodes_b:
            pv = prof[v]
            newload = load_b + pv
            ok = (fill_b < P) & np.all(newload <= caps_b, axis=1)
            assert ok.any(), "packing dead-end; loosen GCN_PAT"
            # min-max relative cell tightness, tie-break on fill
            score = np.where(ok, (newload / caps_b).max(axis=1)
                             + fill_b * 1e-4, np.inf)
            i = int(np.argmin(score))
            t = int(tiles_b[i])
            tile_of[v] = t
            slot_of[v] = int(fill_b[i])
            fill_b[i] += 1
            load_b[i] += pv
        tile_fill[tiles_b] = fill_b.astype(np.int32)
        cell_load[tiles_b] = load_b.astype(np.int64)

    core_of = (tile_of // T_PER_CORE).astype(np.int32)
    tl = (tile_of % T_PER_CORE).astype(np.int64)
    seg_of = (tl // TPS).astype(np.int64)
    assert np.all(seg_of == band_of), "band/seg mismatch"

    # row within the seg's gatherable tensor [SEG_ROWS, HID]
    srow = core_of.astype(np.int64) * (TPS * P) + (tl % TPS) * P + slot_of

    e_src, e_dst = src, dst
    e_cell = tile_of[e_dst].astype(np.int64) * NSEG + seg_of[e_src]
    cell_counts = np.bincount(e_cell, minlength=n_tiles_total * NSEG)
    assert np.all(cell_counts.reshape(-1, NSEG) <= caps_row), \
        f"cell overflow {cell_counts.max()}"
    order_e = np.argsort(e_cell, kind="stable")
    e_src, e_dst = e_src[order_e], e_dst[order_e]
    cell_starts = np.zeros(n_tiles_total * NSEG + 1, dtype=np.int64)
    np.cumsum(cell_counts, out=cell_starts[1:])

    x64 = np.asarray(x, dtype=np.float64)
    xT = (x64 * sqrtdeg[:, None]).T.astype(np.float16)

    per_core = []
    for c in range(N_CORES):
        # per-(tile,seg) padded idx / drel cell arrays at per-cell caps
        gidx_cells = {}
        drel_cells = {}
        base = (c * T_PER_CORE) * NSEG
        for t in range(T_PER_CORE):
            for s in range(NSEG):
                cap = CAPS[t][s] * P
                gi = np.zeros(cap, dtype=np.int16)
                dr = np.full(cap, -1.0, dtype=np.float16)
                cid = base + t * NSEG + s
                a, b = cell_starts[cid], cell_starts[cid + 1]
                n = b - a
                if n:
                    sub = np.argsort(srow[e_src[a:b]], kind="stable")
                    gi[:n] = srow[e_src[a:b]][sub].astype(np.int16)
                    dr[:n] = slot_of[e_dst[a:b]][sub].astype(np.float16)
                gidx_cells[(t, s)] = gi
                drel_cells[(t, s)] = dr

        blocks = []
        for g in range(N_GROUPS):
            for s in range(NSEG):
                stream = np.concatenate(
                    [gidx_cells[(g * G_TILES + i, s)] for i in range(G_TILES)])
                blocks.append(_wrap16(stream))
        gidx_w = np.concatenate(blocks, axis=1)      # [128, IDX_TOT/16]

        drel_cols = np.zeros((P, KTOT), dtype=np.float16)
        for t in range(T_PER_CORE):
            for s in range(NSEG):
                c0 = t * K_T + CELL_BASE[t][s]
                drel_cols[:, c0:c0 + CAPS[t][s]] = \
                    drel_cells[(t, s)].reshape(CAPS[t][s], P).T

        mask = core_of == c
        vids = np.nonzero(mask)[0]
        pos = tl[vids] * P + slot_of[vids]
        x_shard_T = np.zeros((IN_PAD, SHARD), dtype=np.float16)
        x_shard_T[:IN_DIM, pos] = xT[:, vids]
        x_shard_T[IN_DIM, pos] = sqrtdeg[vids].astype(np.float16)

        islot = slot_of[vids]
        itile = tl[vids]
        wscale = np.zeros((P, T_PER_CORE), dtype=np.float32)
        wscale[islot, itile] = (0.9 / deg[vids]).astype(np.float32)
        iscale = np.zeros((P, T_PER_CORE), dtype=np.float32)
        iscale[islot, itile] = (1.0 / deg[vids]).astype(np.float32)
        oscale = np.zeros(SHARD, dtype=np.float32)
        oscale[pos] = (0.9 * dinv[vids]).astype(np.float32)

        per_core.append(dict(
            x_shard_T=x_shard_T,
            gidx=gidx_w,
            drel=np.ascontiguousarray(drel_cols),
            wscale=wscale,
            iscale=iscale,
            oscale=oscale,
        ))
    return per_core, core_of, tl, slot_of


# ------------------------------------------------------------- device kernel
_BUILD_CACHE = {}


def _build(n_layers=N_LAYERS):
    key = n_layers
    if key in _BUILD_CACHE:
        return _BUILD_CACHE[key]
    import concourse.bass as bass
    import concourse.bacc as bacc
    import concourse.tile as tile
    import concourse.mybir as mybir

    F32 = mybir.dt.float32
    F16 = mybir.dt.float16
    I16 = mybir.dt.int16
    AT = mybir.AluOpType
    ts = bass.ts

    nc = bacc.Bacc("TRN2", target_bir_lowering=False, debug=False,
                   num_devices=N_CORES, num_swdge_queues=N_QUEUES)
    if N_QUEUES >= 4:
        # the 4th SWDGE queue only gets a DGE context slot if the unused
        # Activation HWDGE dynamic queue is not declared
        nc.hwdge_engines = type(nc.hwdge_engines)(
            [e for e in nc.hwdge_engines if e.name != "Activation"])
        nc.m.queues = [q for q in nc.m.queues if q.name != "qActDynamicHW"]

    x_in = nc.dram_tensor("x_shard_T", [IN_PAD, SHARD], F16, kind="ExternalInput")
    gidx_in = nc.dram_tensor("gidx", [P, IDX_TOT // 16], I16, kind="ExternalInput")
    drel_in = nc.dram_tensor("drel", [P, KTOT], F16, kind="ExternalInput")
    wscale_in = nc.dram_tensor("wscale", [P, T_PER_CORE], F32, kind="ExternalInput")
    iscale_in = nc.dram_tensor("iscale", [P, T_PER_CORE], F32, kind="ExternalInput")
    iota_in = nc.dram_tensor("iota_mod", [P, K_T * P], F16, kind="ExternalInput")
    ident_in = nc.dram_tensor("ident", [P, P], F16, kind="ExternalInput")
    win_in = nc.dram_tensor("W_in_stack", [P, IN_PAD], F16, kind="ExternalInput")
    wl_in = nc.dram_tensor("Wl_stack", [P, n_layers * HID], F16, kind="ExternalInput")
    wout_in = nc.dram_tensor("W_out_col", [P, 1], F16, kind="ExternalInput")

    out_t = nc.dram_tensor("out_shard", [1, SHARD], F32, kind="ExternalOutput")

    MAXCC = max(max(row) for row in CALL_CH)

    with tile.TileContext(nc) as tc:
        with (
            tc.tile_pool(name="res", bufs=1) as res,
            tc.tile_pool(name="gpool", bufs=4) as gpool,
            tc.tile_pool(name="spool", bufs=2) as spool,
            tc.tile_pool(name="work", bufs=3) as work,
            tc.tile_pool(name="ppool_a", bufs=2, space="PSUM") as ppool_a,
            tc.tile_pool(name="ppool_b", bufs=2, space="PSUM") as ppool_b,
            tc.tile_pool(name="ppool_c", bufs=2, space="PSUM") as ppool_c,
            tc.tile_pool(name="dram", bufs=1, space="DRAM") as dram,
        ):
            gidx_r = res.tile([P, IDX_TOT // 16], I16)
            drel_r = res.tile([P, KTOT], F16)
            wscale_r = res.tile([P, T_PER_CORE], F32)
            iscale_r = res.tile([P, T_PER_CORE], F32)
            iota_r = res.tile([P, K_T * P], F16)
            ident_r = res.tile([P, P], F16)
            win_r = res.tile([P, IN_PAD], F16)
            wl_r = res.tile([P, n_layers * HID], F16)
            wout_r = res.tile([P, 1], F16)
            x0s_r = res.tile([P, SHARD], F16)
            for sb, dr in [(gidx_r, gidx_in), (drel_r, drel_in),
                           (wscale_r, wscale_in), (iscale_r, iscale_in),
                           (iota_r, iota_in), (ident_r, ident_in),
                           (win_r, win_in), (wl_r, wl_in),
                           (wout_r, wout_in)]:
                nc.sync.dma_start(sb[:], dr[:])

            xnext = dram.tile([SHARD, HID], F16)
            xf = [[dram.tile([SEG_ROWS, HID], F16, addr_space="Shared",
                             name=f"xf{i}_{s}") for s in range(NSEG)]
                  for i in range(n_layers)]

            def emit_ag(lidx, s):
                nc.gpsimd.collective_compute(
                    "AllGather", mybir.AluOpType.bypass,
                    replica_groups=[list(range(N_CORES))],
                    ins=[xnext[s * TPS * P:(s + 1) * TPS * P, :]],
                    outs=[xf[lidx][s].opt()])

            # ---- initial projection
            init_scope = nc.enter_named_scope("init", False)
            for t in range(T_PER_CORE):
                xt = work.tile([P, IN_PAD], F16, name="xt")
                for k in range(IN_PAD // P):
                    nc.sync.dma_start(xt[:, ts(k, P)], x_in[ts(k, P), ts(t, P)])
                ps_x = ppool_a.tile([P, P], F32, name="ps_x", tag="ps_agg")
                for k in range(IN_PAD // P):
                    nc.tensor.matmul(
                        out=ps_x[:], lhsT=win_r[:, ts(k, P)], rhs=xt[:, ts(k, P)],
                        start=(k == 0), stop=(k == IN_PAD // P - 1))
                nc.vector.tensor_scalar(
                    out=x0s_r[:, ts(t, P)], in0=ps_x[:],
                    scalar1=ALPHA / 0.9, scalar2=None, op0=AT.mult)
                ps_t = ppool_c.tile([P, P], F16, name="ps_t", tag="ps_t")
                xps = work.tile([P, P], F16, name="xps")
                nc.vector.tensor_copy(xps[:], ps_x[:])
                nc.tensor.matmul(out=ps_t[:], lhsT=xps[:], rhs=ident_r[:],
                                 is_transpose=True)
                xn_sb = work.tile([P, P], F16, name="xn_sb")
                nc.scalar.activation(
                    xn_sb[:], ps_t[:],
                    mybir.ActivationFunctionType.Copy,
                    scale=iscale_r[:, t:t + 1])
                nc.sync.dma_start(xnext[ts(t, P), :], xn_sb[:])
                if (t + 1) % TPS == 0:
                    emit_ag(0, t // TPS)

            nc.leave_named_scope("init", init_scope[0], False)

            # ---- layers
            qctr = 0
            for l in range(n_layers):
                lay_scope = nc.enter_named_scope(f"layer{l}", False)
                beta = BETAS[l]
                xsrc = xf[l]
                gb012 = {}          # (g) -> [gbuf s0, s1, s2]
                gb3 = {}

                def gen_call(g, s):
                    nonlocal qctr
                    cc = CALL_CH[g][s]
                    blk = CALL_OFF[(g, s)] // 16
                    gbuf = gpool.tile([P, MAXCC * P], F16, name=f"gbuf{s}")
                    nc.gpsimd.dma_gather(
                        out_ap=gbuf[:, :cc * P].rearrange("p (c e) -> p c e", c=cc),
                        in_ap=xsrc[s][:],
                        idxs_ap=gidx_r[:, blk:blk + cc * P // 16],
                        num_idxs=cc * P, num_idxs_reg=cc * P, elem_size=HID,
                        single_packet=False, queue_num=qctr % N_QUEUES)
                    qctr += 1
                    return gbuf

                def compute_group(g):
                    gbs = gb012.pop(g) + [gb3.pop(g)]
                    for i in range(G_TILES):
                        t = g * G_TILES + i
                        s_t = spool.tile([P, K_T * P], F16, name="s_t")
                        nc.vector.tensor_tensor(
                            out=s_t[:].rearrange("p (c e) -> p c e", c=K_T),
                            in0=drel_r[:, t * K_T:(t + 1) * K_T]
                                .to_broadcast([P, K_T, P]),
                            in1=iota_r[:].rearrange("p (c e) -> p c e", c=K_T),
                            op=AT.is_equal)
                        xself = work.tile([P, P], F16, name="xself")
                        nc.sync.dma_start(xself[:], xnext[ts(t, P), :])
                        ps_agg = ppool_a.tile([P, P], F32, name="ps_agg")
                        nc.tensor.matmul(out=ps_agg[:], lhsT=xself[:],
                                         rhs=ident_r[:], start=True, stop=False)
                        ch = 0
                        for s in range(NSEG):
                            off = TILE_OFF[g][s][i]
                            for j in range(CAPS[t][s]):
                                nc.tensor.matmul(
                                    out=ps_agg[:],
                                    lhsT=gbs[s][:, ts(off + j, P)],
                                    rhs=s_t[:, ts(ch, P)],
                                    start=False, stop=(ch == K_T - 1))
                                ch += 1
                        h_t = work.tile([P, P], F16, name="h_t")
                        nc.vector.tensor_tensor(
                            out=h_t[:], in0=ps_agg[:], in1=x0s_r[:, ts(t, P)],
                            op=AT.add)
                        ps_d = ppool_b.tile([P, P], F32, name="ps_d")
                        nc.tensor.matmul(out=ps_d[:], lhsT=wl_r[:, ts(l, P)],
                                         rhs=h_t[:], start=True, stop=True)
                        xn_t = work.tile([P, P], F16, name="xn_t")
                        nc.scalar.activation(
                            xn_t[:], ps_d[:],
                            mybir.ActivationFunctionType.Relu, scale=1.0 - beta)
                        if l < n_layers - 1:
                            ps_t2 = ppool_c.tile([P, P], F16, name="ps_t2",
                                                 tag="ps_t")
                            nc.tensor.matmul(out=ps_t2[:], lhsT=xn_t[:],
                                             rhs=ident_r[:], is_transpose=True)
                            xw = work.tile([P, P], F16, name="xw")
                            nc.scalar.activation(
                                xw[:], ps_t2[:],
                                mybir.ActivationFunctionType.Copy,
                                scale=wscale_r[:, t:t + 1])
                            nc.sync.dma_start(xnext[ts(t, P), :], xw[:])
                        else:
                            ps_o = ppool_b.tile([1, P], F32, name="ps_o",
                                                tag="ps_d")
                            nc.tensor.matmul(out=ps_o[:], lhsT=wout_r[:],
                                             rhs=xn_t[:], start=True, stop=True)
                            ot = work.tile([1, P], F32, name="ot")
                            nc.scalar.copy(ot[:], ps_o[:])
                            nc.sync.dma_start(out_t[:, ts(t, P)], ot[:])

                # pipelined steps: gen s012(g); DEFER3 groups later gen s3
                # and compute, so the seg-3 AllGather of the previous layer
                # hides behind the early s012 drains
                for step in range(N_GROUPS + DEFER3):
                    if step < N_GROUPS:
                        gb012[step] = [gen_call(step, s) for s in range(3)]
                    if step >= DEFER3:
                        g = step - DEFER3
                        gb3[g] = gen_call(g, 3)
                        compute_group(g)
                        if l < n_layers - 1:
                            if g in (6, 11, 16):
                                emit_ag(l + 1, g // 5 - 1)
                if l < n_layers - 1:
                    emit_ag(l + 1, 3)
                nc.leave_named_scope(f"layer{l}", lay_scope[0], False)

    nc.compile()
    _BUILD_CACHE[key] = nc
    return nc


# ------------------------------------------------------------------ runner
def kernel(x, edge_index, edge_weight, W_in, b_in, W_layers, W_out, b_out):
    import concourse.bass_utils as bass_utils

    x = np.asarray(x)
    per_core, core_of, tl, slot_of = _preprocess(x, edge_index)

    W_in = np.asarray(W_in, np.float32)
    b_in = np.asarray(b_in, np.float32)
    W_layers = np.asarray(W_layers, np.float32)
    W_out = np.asarray(W_out, np.float32)
    b_out = np.asarray(b_out, np.float32)

    win_full = np.zeros((IN_PAD, HID), dtype=np.float32)
    win_full[:IN_DIM] = W_in
    win_full[IN_DIM] = b_in
    win_stack = win_full.reshape(IN_PAD // P, P, HID).transpose(1, 0, 2) \
                        .reshape(P, IN_PAD).astype(np.float16)
    eye = np.eye(HID, dtype=np.float64)
    wl_stack = np.concatenate(
        [eye + BETAS[l] / (1.0 - BETAS[l]) * W_layers[l].astype(np.float64)
         for l in range(N_LAYERS)],
        axis=1).astype(np.float16)
    iota_mod = np.broadcast_to(
        np.tile(np.arange(P, dtype=np.float32), K_T),
        (P, K_T * P)).astype(np.float16)
    ident = np.eye(P, dtype=np.float16)

    n_layers = int(os.environ.get('GCN_LAYERS', str(N_LAYERS)))
    in_maps = []
    for c in range(N_CORES):
        d = per_core[c]
        in_maps.append({
            "x_shard_T": d["x_shard_T"],
            "gidx": d["gidx"],
            "drel": d["drel"],
            "wscale": d["wscale"],
            "iscale": d["iscale"],
            "iota_mod": np.ascontiguousarray(iota_mod),
            "ident": ident,
            "W_in_stack": np.ascontiguousarray(win_stack),
            "Wl_stack": np.ascontiguousarray(wl_stack),
            "W_out_col": W_out.reshape(P, 1).astype(np.float16),
        })

    nc = _build(n_layers)
    trace = bool(int(os.environ.get("GCN_TRACE", "0")))
    res = bass_utils.run_bass_kernel_spmd(
        nc, in_maps, core_ids=list(range(N_CORES)), trace=trace)
    kernel.last_results = res

    out = np.zeros((N_NODES, 1), dtype=np.float32)
    pos = tl * P + slot_of
    for c in range(N_CORES):
        mask = core_of == c
        raw = res.results[c]["out_shard"][0]
        osc = per_core[c]["oscale"]
        out[mask, 0] = raw[pos[mask]] * osc[pos[mask]] + b_out[0]
    return out



# revision 3
# speedup vs baseline: 1.0036x; 1.0036x over previous
"""GCN2Net Trainium2 kernel (8-core SPMD), v5.

v2 -> v3:
- Rotated per-cell chunk caps PAT[(s - t) % 4] (default [5,5,4,4]): 18 chunks
  per tile instead of 20 (-10% gather packets), enforced by a cell-aware
  vector bin-pack on the host.
- Seg-3 AllGather split into 5 per-group parts issued as their tiles finish,
  so the layer-boundary collective is ~1.6MB instead of 6.5MB.
- Seg-3 gather calls deferred by one group so their wait on the (split)
  AllGather never stalls the gather queue.
"""
import math
import os
import numpy as np

P = 128
N_CORES = 8
N_NODES = 100000
IN_DIM = 512
HID = 128
N_LAYERS = 8
ALPHA = 0.1
THETA = 0.5

T_PER_CORE = 100
SHARD = T_PER_CORE * P            # 12800
NSEG = 4
TPS = T_PER_CORE // NSEG          # 25 tiles per seg
SEG_ROWS = N_CORES * TPS * P      # 25600 (< 32768)
G_TILES = 5                       # tiles per gather group
N_GROUPS = T_PER_CORE // G_TILES  # 20
GPS = TPS // G_TILES              # 5 groups per seg
IN_PAD = 640                      # x rows 0-511, b-row 512, zero pad
N_QUEUES = int(os.environ.get("GCN_QUEUES", "4"))
PAT = [int(c) for c in os.environ.get("GCN_PAT", "5444")]
assert len(PAT) == NSEG

BETAS = [math.log(THETA / (i + 1) + 1.0) for i in range(N_LAYERS)]

# static chunk geometry
CAPS = [[PAT[(s - t) % NSEG] for s in range(NSEG)] for t in range(T_PER_CORE)]
K_T = sum(CAPS[0])                                  # chunks per tile (const)
assert all(sum(CAPS[t]) == K_T for t in range(T_PER_CORE))
KTOT = T_PER_CORE * K_T
# chunk base of (t, s) within tile t's chunk list
CELL_BASE = [[sum(CAPS[t][:s]) for s in range(NSEG)] for t in range(T_PER_CORE)]
# chunks per gather call (g, s)
CALL_CH = [[sum(CAPS[g * G_TILES + i][s] for i in range(G_TILES))
            for s in range(NSEG)] for g in range(N_GROUPS)]
# tile offset (in chunks) within call (g, s)
TILE_OFF = [[[sum(CAPS[g * G_TILES + j][s] for j in range(i))
              for i in range(G_TILES)] for s in range(NSEG)]
            for g in range(N_GROUPS)]
# idx-stream block offsets (in idxs), canonical (g, s) order
CALL_OFF = {}
_off = 0
for _g in range(N_GROUPS):
    for _s in range(NSEG):
        CALL_OFF[(_g, _s)] = _off
        _off += CALL_CH[_g][_s] * P
IDX_TOT = _off                                       # == KTOT * P
DEFER3 = 3                        # defer seg-3 gather calls by this many groups


# ----------------------------------------------------------------- host prep
def _wrap16(stream):
    n = stream.shape[0]
    wrap = stream.reshape(n // 16, 16).T.astype(np.int16)
    out = np.zeros((P, n // 16), dtype=np.int16)
    for rg in range(8):
        out[rg * 16:(rg + 1) * 16] = wrap
    return out


def _preprocess(x, edge_index):
    import heapq

    src = np.asarray(edge_index[0], dtype=np.int64)
    dst = np.asarray(edge_index[1], dtype=np.int64)

    deg = np.bincount(dst, minlength=N_NODES).astype(np.float64) + 1.0
    dinv = 1.0 / np.sqrt(deg)
    sqrtdeg = np.sqrt(deg)
    cnt = np.bincount(dst, minlength=N_NODES).astype(np.int64)

    n_tiles_total = N_CORES * T_PER_CORE
    order = np.argsort(-cnt, kind="stable")

    band_of = np.empty(N_NODES, dtype=np.int64)
    band_of[order] = np.arange(N_NODES, dtype=np.int64) % NSEG

    prof = np.zeros((N_NODES, NSEG), dtype=np.int64)
    np.add.at(prof, (dst, band_of[src]), 1)

    caps_row = np.array(
        [[CAPS[t % T_PER_CORE][s] * P for s in range(NSEG)]
         for t in range(n_tiles_total)], dtype=np.int64)

    tile_of = np.empty(N_NODES, dtype=np.int32)
    slot_of = np.empty(N_NODES, dtype=np.int32)
    tile_fill = np.zeros(n_tiles_total, dtype=np.int32)
    cell_load = np.zeros((n_tiles_total, NSEG), dtype=np.int64)
    for b in range(NSEG):
        tiles_b = np.array([c * T_PER_CORE + b * TPS + j
                            for c in range(N_CORES) for j in range(TPS)])
        caps_b = caps_row[tiles_b].astype(np.float64)
        load_b = np.zeros((len(tiles_b), NSEG))
        fill_b = np.zeros(len(tiles_b))
        nodes_b = order[band_of[order] == b]
        for v in nodes_b:
            pv = prof[v]
            newload = load_b + pv
            ok = (fill_b < P) & np.all(newload <= caps_b, axis=1)
            assert ok.any(), "packing dead-end; loosen GCN_PAT"
            # min-max relative cell tightness, tie-break on fill
            score = np.where(ok, (newload / caps_b).max(axis=1)
                             + fill_b * 1e-4, np.inf)
            i = int(np.argmin(score))
            t = int(tiles_b[i])
            tile_of[v] = t
            slot_of[v] = int(fill_b[i])
            fill_b[i] += 1
            load_b[i] += pv
        tile_fill[tiles_b] = fill_b.astype(np.int32)
        cell_load[tiles_b] = load_b.astype(np.int64)

    core_of = (tile_of // T_PER_CORE).astype(np.int32)
    tl = (tile_of % T_PER_CORE).astype(np.int64)
    seg_of = (tl // TPS).astype(np.int64)
    assert np.all(seg_of == band_of), "band/seg mismatch"

    # row within the seg's gatherable tensor [SEG_ROWS, HID]
    srow = core_of.astype(np.int64) * (TPS * P) + (tl % TPS) * P + slot_of

    e_src, e_dst = src, dst
    e_cell = tile_of[e_dst].astype(np.int64) * NSEG + seg_of[e_src]
    cell_counts = np.bincount(e_cell, minlength=n_tiles_total * NSEG)
    assert np.all(cell_counts.reshape(-1, NSEG) <= caps_row), \
        f"cell overflow {cell_counts.max()}"
    order_e = np.argsort(e_cell, kind="stable")
    e_src, e_dst = e_src[order_e], e_dst[order_e]
    cell_starts = np.zeros(n_tiles_total * NSEG + 1, dtype=np.int64)
    np.cumsum(cell_counts, out=cell_starts[1:])

    x64 = np.asarray(x, dtype=np.float64)
    xT = (x64 * sqrtdeg[:, None]).T.astype(np.float16)

    per_core = []
    for c in range(N_CORES):
        # per-(tile,seg) padded idx / drel cell arrays at per-cell caps
        gidx_cells = {}
        drel_cells = {}
        base = (c * T_PER_CORE) * NSEG
        for t in range(T_PER_CORE):
            for s in range(NSEG):
                cap = CAPS[t][s] * P
                gi = np.zeros(cap, dtype=np.int16)
                dr = np.full(cap, -1.0, dtype=np.float16)
                cid = base + t * NSEG + s
                a, b = cell_starts[cid], cell_starts[cid + 1]
                n = b - a
                if n:
                    sub = np.argsort(srow[e_src[a:b]], kind="stable")
                    gi[:n] = srow[e_src[a:b]][sub].astype(np.int16)
                    dr[:n] = slot_of[e_dst[a:b]][sub].astype(np.float16)
                gidx_cells[(t, s)] = gi
                drel_cells[(t, s)] = dr

        blocks = []
        for g in range(N_GROUPS):
            for s in range(NSEG):
                stream = np.concatenate(
                    [gidx_cells[(g * G_TILES + i, s)] for i in range(G_TILES)])
                blocks.append(_wrap16(stream))
        gidx_w = np.concatenate(blocks, axis=1)      # [128, IDX_TOT/16]

        drel_cols = np.zeros((P, KTOT), dtype=np.float16)
        for t in range(T_PER_CORE):
            for s in range(NSEG):
                c0 = t * K_T + CELL_BASE[t][s]
                drel_cols[:, c0:c0 + CAPS[t][s]] = \
                    drel_cells[(t, s)].reshape(CAPS[t][s], P).T

        mask = core_of == c
        vids = np.nonzero(mask)[0]
        pos = tl[vids] * P + slot_of[vids]
        x_shard_T = np.zeros((IN_PAD, SHARD), dtype=np.float16)
        x_shard_T[:IN_DIM, pos] = xT[:, vids]
        x_shard_T[IN_DIM, pos] = sqrtdeg[vids].astype(np.float16)

        islot = slot_of[vids]
        itile = tl[vids]
        wscale = np.zeros((P, T_PER_CORE), dtype=np.float32)
        wscale[islot, itile] = (0.9 / deg[vids]).astype(np.float32)
        iscale = np.zeros((P, T_PER_CORE), dtype=np.float32)
        iscale[islot, itile] = (1.0 / deg[vids]).astype(np.float32)
        oscale = np.zeros(SHARD, dtype=np.float32)
        oscale[pos] = (0.9 * dinv[vids]).astype(np.float32)

        per_core.append(dict(
            x_shard_T=x_shard_T,
            gidx=gidx_w,
            drel=np.ascontiguousarray(drel_cols),
            wscale=wscale,
            iscale=iscale,
            oscale=oscale,
        ))
    return per_core, core_of, tl, slot_of


# ------------------------------------------------------------- device kernel
_BUILD_CACHE = {}


def _build(n_layers=N_LAYERS):
    key = n_layers
    if key in _BUILD_CACHE:
        return _BUILD_CACHE[key]
    import concourse.bass as bass
    import concourse.bacc as bacc
    import concourse.tile as tile
    import concourse.mybir as mybir

    F32 = mybir.dt.float32
    F16 = mybir.dt.float16
    I16 = mybir.dt.int16
    AT = mybir.AluOpType
    ts = bass.ts

    nc = bacc.Bacc("TRN2", target_bir_lowering=False, debug=False,
                   num_devices=N_CORES, num_swdge_queues=N_QUEUES,
                   dynamic_dma_scratch_size=int(os.environ.get("GCN_SCRATCH", "16384")))
    if N_QUEUES >= 4:
        # the 4th SWDGE queue only gets a DGE context slot if the unused
        # Activation HWDGE dynamic queue is not declared
        nc.hwdge_engines = type(nc.hwdge_engines)(
            [e for e in nc.hwdge_engines if e.name != "Activation"])
        nc.m.queues = [q for q in nc.m.queues if q.name != "qActDynamicHW"]

    x_in = nc.dram_tensor("x_shard_T", [IN_PAD, SHARD], F16, kind="ExternalInput")
    gidx_in = nc.dram_tensor("gidx", [P, IDX_TOT // 16], I16, kind="ExternalInput")
    drel_in = nc.dram_tensor("drel", [P, KTOT], F16, kind="ExternalInput")
    wscale_in = nc.dram_tensor("wscale", [P, T_PER_CORE], F32, kind="ExternalInput")
    iscale_in = nc.dram_tensor("iscale", [P, T_PER_CORE], F32, kind="ExternalInput")
    iota_in = nc.dram_tensor("iota_mod", [P, K_T * P], F16, kind="ExternalInput")
    ident_in = nc.dram_tensor("ident", [P, P], F16, kind="ExternalInput")
    win_in = nc.dram_tensor("W_in_stack", [P, IN_PAD], F16, kind="ExternalInput")
    wl_in = nc.dram_tensor("Wl_stack", [P, n_layers * HID], F16, kind="ExternalInput")
    wout_in = nc.dram_tensor("W_out_col", [P, 1], F16, kind="ExternalInput")

    out_t = nc.dram_tensor("out_shard", [1, SHARD], F32, kind="ExternalOutput")

    MAXCC = max(max(row) for row in CALL_CH)

    with tile.TileContext(nc) as tc:
        with (
            tc.tile_pool(name="res", bufs=1) as res,
            tc.tile_pool(name="gpool", bufs=4) as gpool,
            tc.tile_pool(name="spool", bufs=2) as spool,
            tc.tile_pool(name="work", bufs=3) as work,
            tc.tile_pool(name="ppool_a", bufs=2, space="PSUM") as ppool_a,
            tc.tile_pool(name="ppool_b", bufs=2, space="PSUM") as ppool_b,
            tc.tile_pool(name="ppool_c", bufs=2, space="PSUM") as ppool_c,
            tc.tile_pool(name="dram", bufs=1, space="DRAM") as dram,
        ):
            gidx_r = res.tile([P, IDX_TOT // 16], I16)
            drel_r = res.tile([P, KTOT], F16)
            wscale_r = res.tile([P, T_PER_CORE], F32)
            iscale_r = res.tile([P, T_PER_CORE], F32)
            iota_r = res.tile([P, K_T * P], F16)
            ident_r = res.tile([P, P], F16)
            win_r = res.tile([P, IN_PAD], F16)
            wl_r = res.tile([P, n_layers * HID], F16)
            wout_r = res.tile([P, 1], F16)
            x0s_r = res.tile([P, SHARD], F16)
            for sb, dr in [(gidx_r, gidx_in), (drel_r, drel_in),
                           (wscale_r, wscale_in), (iscale_r, iscale_in),
                           (iota_r, iota_in), (ident_r, ident_in),
                           (win_r, win_in), (wl_r, wl_in),
                           (wout_r, wout_in)]:
                nc.sync.dma_start(sb[:], dr[:])

            xnext = dram.tile([SHARD, HID], F16)
            xf = [[dram.tile([SEG_ROWS, HID], F16, addr_space="Shared",
                             name=f"xf{i}_{s}") for s in range(NSEG)]
                  for i in range(n_layers)]

            def emit_ag(lidx, s):
                nc.gpsimd.collective_compute(
                    "AllGather", mybir.AluOpType.bypass,
                    replica_groups=[list(range(N_CORES))],
                    ins=[xnext[s * TPS * P:(s + 1) * TPS * P, :]],
                    outs=[xf[lidx][s].opt()])

            # ---- initial projection
            init_scope = nc.enter_named_scope("init", False)
            for t in range(T_PER_CORE):
                xt = work.tile([P, IN_PAD], F16, name="xt")
                for k in range(IN_PAD // P):
                    nc.sync.dma_start(xt[:, ts(k, P)], x_in[ts(k, P), ts(t, P)])
                ps_x = ppool_a.tile([P, P], F32, name="ps_x", tag="ps_agg")
                for k in range(IN_PAD // P):
                    nc.tensor.matmul(
                        out=ps_x[:], lhsT=win_r[:, ts(k, P)], rhs=xt[:, ts(k, P)],
                        start=(k == 0), stop=(k == IN_PAD // P - 1))
                nc.vector.tensor_scalar(
                    out=x0s_r[:, ts(t, P)], in0=ps_x[:],
                    scalar1=ALPHA / 0.9, scalar2=None, op0=AT.mult)
                ps_t = ppool_c.tile([P, P], F16, name="ps_t", tag="ps_t")
                xps = work.tile([P, P], F16, name="xps")
                nc.vector.tensor_copy(xps[:], ps_x[:])
                nc.tensor.matmul(out=ps_t[:], lhsT=xps[:], rhs=ident_r[:],
                                 is_transpose=True)
                xn_sb = work.tile([P, P], F16, name="xn_sb")
                nc.scalar.activation(
                    xn_sb[:], ps_t[:],
                    mybir.ActivationFunctionType.Copy,
                    scale=iscale_r[:, t:t + 1])
                nc.sync.dma_start(xnext[ts(t, P), :], xn_sb[:])
                if (t + 1) % TPS == 0:
                    emit_ag(0, t // TPS)

            nc.leave_named_scope("init", init_scope[0], False)

            # ---- layers
            qctr = 0
            for l in range(n_layers):
                lay_scope = nc.enter_named_scope(f"layer{l}", False)
                beta = BETAS[l]
                xsrc = xf[l]
                gb012 = {}          # (g) -> [gbuf s0, s1, s2]
                gb3 = {}

                def gen_call(g, s):
                    nonlocal qctr
                    cc = CALL_CH[g][s]
                    blk = CALL_OFF[(g, s)] // 16
                    gbuf = gpool.tile([P, MAXCC * P], F16, name=f"gbuf{s}")
                    nc.gpsimd.dma_gather(
                        out_ap=gbuf[:, :cc * P].rearrange("p (c e) -> p c e", c=cc),
                        in_ap=xsrc[s][:],
                        idxs_ap=gidx_r[:, blk:blk + cc * P // 16],
                        num_idxs=cc * P, num_idxs_reg=cc * P, elem_size=HID,
                        single_packet=bool(int(os.environ.get("GCN_SP","0"))), queue_num=qctr % N_QUEUES)
                    qctr += 1
                    return gbuf

                def compute_group(g):
                    gbs = gb012.pop(g) + [gb3.pop(g)]
                    for i in range(G_TILES):
                        t = g * G_TILES + i
                        s_t = spool.tile([P, K_T * P], F16, name="s_t")
                        nc.vector.tensor_tensor(
                            out=s_t[:].rearrange("p (c e) -> p c e", c=K_T),
                            in0=drel_r[:, t * K_T:(t + 1) * K_T]
                                .to_broadcast([P, K_T, P]),
                            in1=iota_r[:].rearrange("p (c e) -> p c e", c=K_T),
                            op=AT.is_equal)
                        xself = work.tile([P, P], F16, name="xself")
                        nc.sync.dma_start(xself[:], xnext[ts(t, P), :])
                        ps_agg = ppool_a.tile([P, P], F32, name="ps_agg")
                        nc.tensor.matmul(out=ps_agg[:], lhsT=xself[:],
                                         rhs=ident_r[:], start=True, stop=False)
                        ch = 0
                        for s in range(NSEG):
                            off = TILE_OFF[g][s][i]
                            for j in range(CAPS[t][s]):
                                nc.tensor.matmul(
                                    out=ps_agg[:],
                                    lhsT=gbs[s][:, ts(off + j, P)],
                                    rhs=s_t[:, ts(ch, P)],
                                    start=False, stop=(ch == K_T - 1))
                                ch += 1
                        h_t = work.tile([P, P], F16, name="h_t")
                        nc.vector.tensor_tensor(
                            out=h_t[:], in0=ps_agg[:], in1=x0s_r[:, ts(t, P)],
                            op=AT.add)
                        ps_d = ppool_b.tile([P, P], F32, name="ps_d")
                        nc.tensor.matmul(out=ps_d[:], lhsT=wl_r[:, ts(l, P)],
                                         rhs=h_t[:], start=True, stop=True)
                        xn_t = work.tile([P, P], F16, name="xn_t")
                        nc.scalar.activation(
                            xn_t[:], ps_d[:],
                            mybir.ActivationFunctionType.Relu, scale=1.0 - beta)
                        if l < n_layers - 1:
                            ps_t2 = ppool_c.tile([P, P], F16, name="ps_t2",
                                                 tag="ps_t")
                            nc.tensor.matmul(out=ps_t2[:], lhsT=xn_t[:],
                                             rhs=ident_r[:], is_transpose=True)
                            xw = work.tile([P, P], F16, name="xw")
                            nc.scalar.activation(
                                xw[:], ps_t2[:],
                                mybir.ActivationFunctionType.Copy,
                                scale=wscale_r[:, t:t + 1])
                            nc.sync.dma_start(xnext[ts(t, P), :], xw[:])
                        else:
                            ps_o = ppool_b.tile([1, P], F32, name="ps_o",
                                                tag="ps_d")
                            nc.tensor.matmul(out=ps_o[:], lhsT=wout_r[:],
                                             rhs=xn_t[:], start=True, stop=True)
                            ot = work.tile([1, P], F32, name="ot")
                            nc.scalar.copy(ot[:], ps_o[:])
                            nc.sync.dma_start(out_t[:, ts(t, P)], ot[:])

                # pipelined steps: gen s012(g); DEFER3 groups later gen s3
                # and compute, so the seg-3 AllGather of the previous layer
                # hides behind the early s012 drains
                for step in range(N_GROUPS + DEFER3):
                    if step < N_GROUPS:
                        gb012[step] = [gen_call(step, s) for s in range(3)]
                    if step >= DEFER3:
                        g = step - DEFER3
                        gb3[g] = gen_call(g, 3)
                        compute_group(g)
                        if l < n_layers - 1:
                            if g in (6, 11, 16):
                                emit_ag(l + 1, g // 5 - 1)
                if l < n_layers - 1:
                    emit_ag(l + 1, 3)
                nc.leave_named_scope(f"layer{l}", lay_scope[0], False)

    nc.compile()
    _BUILD_CACHE[key] = nc
    return nc


# ------------------------------------------------------------------ runner
def kernel(x, edge_index, edge_weight, W_in, b_in, W_layers, W_out, b_out):
    import concourse.bass_utils as bass_utils

    x = np.asarray(x)
    per_core, core_of, tl, slot_of = _preprocess(x, edge_index)

    W_in = np.asarray(W_in, np.float32)
    b_in = np.asarray(b_in, np.float32)
    W_layers = np.asarray(W_layers, np.float32)
    W_out = np.asarray(W_out, np.float32)
    b_out = np.asarray(b_out, np.float32)

    win_full = np.zeros((IN_PAD, HID), dtype=np.float32)
    win_full[:IN_DIM] = W_in
    win_full[IN_DIM] = b_in
    win_stack = win_full.reshape(IN_PAD // P, P, HID).transpose(1, 0, 2) \
                        .reshape(P, IN_PAD).astype(np.float16)
    eye = np.eye(HID, dtype=np.float64)
    wl_stack = np.concatenate(
        [eye + BETAS[l] / (1.0 - BETAS[l]) * W_layers[l].astype(np.float64)
         for l in range(N_LAYERS)],
        axis=1).astype(np.float16)
    iota_mod = np.broadcast_to(
        np.tile(np.arange(P, dtype=np.float32), K_T),
        (P, K_T * P)).astype(np.float16)
    ident = np.eye(P, dtype=np.float16)

    n_layers = int(os.environ.get('GCN_LAYERS', str(N_LAYERS)))
    in_maps = []
    for c in range(N_CORES):
        d = per_core[c]
        in_maps.append({
            "x_shard_T": d["x_shard_T"],
            "gidx": d["gidx"],
            "drel": d["drel"],
            "wscale": d["wscale"],
            "iscale": d["iscale"],
            "iota_mod": np.ascontiguousarray(iota_mod),
            "ident": ident,
            "W_in_stack": np.ascontiguousarray(win_stack),
            "Wl_stack": np.ascontiguousarray(wl_stack),
            "W_out_col": W_out.reshape(P, 1).astype(np.float16),
        })

    nc = _build(n_layers)
    trace = bool(int(os.environ.get("GCN_TRACE", "0")))
    res = bass_utils.run_bass_kernel_spmd(
        nc, in_maps, core_ids=list(range(N_CORES)), trace=trace)
    kernel.last_results = res

    out = np.zeros((N_NODES, 1), dtype=np.float32)
    pos = tl * P + slot_of
    for c in range(N_CORES):
        mask = core_of == c
        raw = res.results[c]["out_shard"][0]
        osc = per_core[c]["oscale"]
        out[mask, 0] = raw[pos[mask]] * osc[pos[mask]] + b_out[0]
    return out

